# revision 15
# baseline (speedup 1.0000x reference)
"""Trainium2 Bass kernel for a two-LSTM premise/hypothesis classifier.

Model (per reference):
  emb_p = emb[premise]; emb_h = emb[hypothesis]              # [B,T,E]
  premise LSTM (h0=c0=0) -> masked-blend cell -> c_last
  hypothesis LSTM (h0=0, c0=c_last) -> masked-blend hidden -> h_last
  logits = [h_last | similarity] @ fc_W.T + fc_b; out = log_softmax

Sharding: data-parallel over batch, 128 -> 16 per core on 8 cores.

Structure (v2):
  - tokens processed in t-major order so each recurrence step reads a
    contiguous 16-col slice of the precomputed input projection (xg).
  - xg stored as fp8e4m3 in per-512-token chunk tiles; chunk-level
    dependencies let each LSTM's recurrence start as soon as chunk 0 is
    projected, and let the hypothesis-side gather/projection stream into
    the premise recurrence's tensor-engine idle gaps (filler thunks with
    min-step staggering + deadline forcing, so the in-order engine queues
    neither stall nor miss a read-after-write emission ordering).
  - recurrence: per step 3 gate blocks (if/g/o) in double-buffered PSUM;
    xg enters via one fp8 identity matmul per block; 64 fp16 [128,128]
    weight-stationary matmuls (LDWEIGHTS-bound, ~27ns/pair).
  - tail: sigmoid/tanh ACTs; f16 outputs feed the h16 multiply (DVE 2x).
"""

import os
import sys
from collections import deque

import numpy as np

for _p in ("/opt/trn_rl_repo", "/root/.axon_site/_ro/trn_rl_repo"):
    if _p not in sys.path and os.path.isdir(_p):
        sys.path.insert(0, _p)

import concourse.bass as bass  # noqa: E402
from concourse import bacc  # noqa: E402
import concourse.mybir as mybir  # noqa: E402
import concourse.tile as tile  # noqa: E402
from concourse.bass import IndirectOffsetOnAxis, AP  # noqa: E402
from concourse.bass_utils import run_bass_kernel_spmd  # noqa: E402
from concourse.masks import make_identity  # noqa: E402

F32 = mybir.dt.float32
F16 = mybir.dt.float16
F8 = mybir.dt.float8e4
I32 = mybir.dt.int32
U8 = mybir.dt.uint8

B, T, V, E, H, C = 128, 200, 50000, 300, 512, 3
NCORES = 8
BL = B // NCORES          # 16 local batch
G4 = 4 * H                # 2048 gates
NM = G4 // 128            # 16 gate m-tiles
NKH = H // 128            # 4 h-dim k-tiles
EP = 384                  # E padded to 3*128
EK = EP // 128            # 3 E k-tiles
NTOK = BL * T             # 3200 tokens per core
NG = NTOK // 128          # 25 gather tiles (128 tokens each, t-major)
TTG = 128 // BL           # 8 t-values per gather tile
CH = 512                  # tokens per xg chunk
NCH = (NTOK + CH - 1) // CH   # 7 chunks (6x512 + 128)
SPC = CH // BL            # 32 steps per chunk
AF = mybir.ActivationFunctionType
ALU = mybir.AluOpType


def _chunk_ntok(c):
    return min(CH, NTOK - c * CH)


class Filler:
    """FIFO queue of small emission thunks drained between recurrence steps.

    Items: (min_step, deadline, cost_us, fn) in GLOBAL step units
    (premise t = 0..199, hypothesis t = 200..399).  Head is emitted when
    t >= min_step and the per-step cost budget allows; a head whose
    deadline is due is emitted unconditionally (emission order = program
    order, so writers must be emitted before their readers).
    """

    def __init__(self):
        self.q = deque()

    def add(self, min_step, deadline, cost, fn):
        self.q.append((min_step, deadline, cost, fn))

    def drain(self, t, budget=0.85):
        spent = 0.0
        while self.q:
            ms, dl, cost, fn = self.q[0]
            due = t >= dl
            if not due:
                if ms > t:
                    break
                if spent + cost > budget:
                    break
            self.q.popleft()
            fn()
            spent += cost

    def drain_all(self):
        while self.q:
            _, _, _, fn = self.q.popleft()
            fn()


def _tok_dma(nc, tokt, tok_dram):
    """tokt[128, NG] int32, t-major: tokt[tt*16+b, g] = tok[b, g*8+tt]."""
    for tt in range(TTG):
        nc.sync.dma_start(
            out=tokt[tt * BL:(tt + 1) * BL, :],
            in_=tok_dram[:, tt::TTG],
        )


def _load_proj_weights(nc, pools, wih_dram, wihT, ident):
    thunks = []
    for m in range(NM):
        def fn(m=m):
            w32 = pools["w32"].tile([128, EP], F32, tag="w32")
            nc.gpsimd.memset(w32[:, E:EP], 0.0)
            nc.sync.dma_start(out=w32[:, :E], in_=wih_dram[m * 128:(m + 1) * 128, :])
            for k in range(EK):
                tp = pools["pj"].tile([128, 128], F32, tag="pj")
                nc.tensor.transpose(
                    out=tp[:], in_=w32[:, k * 128:(k + 1) * 128], identity=ident[:]
                )
                nc.vector.tensor_copy(
                    out=wihT[:, k * G4 + m * 128:k * G4 + (m + 1) * 128], in_=tp[:]
                )
        thunks.append(fn)
    return thunks


def _load_rec_weights(nc, pools, whh_dram, whhT, ident):
    thunks = []
    for m in range(NM):
        def fn(m=m):
            w32 = pools["w32"].tile([128, H], F32, tag="w32")
            nc.sync.dma_start(out=w32[:], in_=whh_dram[m * 128:(m + 1) * 128, :])
            for k in range(NKH):
                tp = pools["pj"].tile([128, 128], F32, tag="pj")
                nc.tensor.transpose(
                    out=tp[:], in_=w32[:, k * 128:(k + 1) * 128], identity=ident[:]
                )
                nc.vector.tensor_copy(
                    out=whhT[:, k * G4 + m * 128:k * G4 + (m + 1) * 128], in_=tp[:]
                )
        thunks.append(fn)
    return thunks


def _load_bias(nc, pools, bih_dram, bhh_dram, ident, tag):
    """bias[128, 16] f32: col m holds (bih+bhh)[m*128 : (m+1)*128]."""
    b0 = pools["w32"].tile([16, 128], F32, tag="b0")
    b1 = pools["w32"].tile([16, 128], F32, tag="b1")
    nc.sync.dma_start(out=b0[:], in_=bih_dram.rearrange("(m x) -> m x", m=NM))
    nc.sync.dma_start(out=b1[:], in_=bhh_dram.rearrange("(m x) -> m x", m=NM))
    bs = pools["w32"].tile([16, 128], F32, tag="bs")
    nc.vector.tensor_add(out=bs[:], in0=b0[:], in1=b1[:])
    bps = pools["pj"].tile([128, 16], F32, tag="pj")
    nc.tensor.transpose(out=bps[:], in_=bs[:], identity=ident[:16, :16])
    bias = pools["persist"].tile([128, 16], F32, tag=tag)
    nc.vector.tensor_copy(out=bias[:], in_=bps[:])
    return bias


def _mask_chunk_thunks(nc, pools, tok_dram, mask):
    """mask[128, 3200] uint8, t-major: col t*16+b = (tok[b,t] != 0).
    One thunk per 512-col chunk."""
    thunks = []
    for cc in range(NCH):
        nw = _chunk_ntok(cc)
        nt = nw // BL

        def fn(cc=cc, nw=nw, nt=nt):
            sc = pools["mscratch"].tile([128, CH], I32, tag="mscratch")
            nc.sync.dma_start(
                out=sc[0:1, :nw].rearrange("p (t b) -> p t b", t=nt),
                in_=tok_dram[:, cc * SPC:cc * SPC + nt].rearrange("b t -> t b")[None],
            )
            nc.gpsimd.partition_broadcast(sc[:, :nw], sc[0:1, :nw])
            nc.vector.tensor_scalar(
                out=mask[:, cc * CH:cc * CH + nw], in0=sc[:, :nw],
                scalar1=0, scalar2=None, op0=ALU.not_equal,
            )
        thunks.append(fn)
    return thunks


def _gather_chunk_thunks(nc, pools, tokt, emb_dram, ident16, rhsT, c, nw):
    """Thunks gathering+transposing one <=512-token chunk into rhsT.

    rhsT[:, k*512 + gg*128 + q] = emb[token(c*512+gg*128+q)][k*128 + row].
    Returns (dma_thunks, cast_thunks, trans_thunks).
    """
    ngg = nw // 128
    e32s, e16s = {}, {}
    dmas, casts, trans = [], [], []
    for gg in range(ngg):
        g = c * 4 + gg

        def dma(gg=gg, g=g):
            e32 = pools["emb32"].tile([128, E], F32, tag="emb32")
            e32s[gg] = e32
            nc.gpsimd.indirect_dma_start(
                out=e32[:],
                out_offset=None,
                in_=emb_dram[:],
                in_offset=IndirectOffsetOnAxis(ap=tokt[:, g:g + 1], axis=0),
            )

        def cast(gg=gg):
            e16 = pools["emb16"].tile([128, EP], F16, tag="emb16")
            e16s[gg] = e16
            nc.vector.memset(e16[:, E:EP], 0.0)
            nc.vector.tensor_copy(out=e16[:, :E], in_=e32s.pop(gg)[:])

        def tr(gg=gg):
            e16 = e16s.pop(gg)
            for k in range(EK):
                tp = pools["pj"].tile([128, 128], F16, tag="pj")
                nc.tensor.transpose(
                    out=tp[:], in_=e16[:, k * 128:(k + 1) * 128], identity=ident16[:]
                )
                nc.vector.tensor_copy(
                    out=rhsT[:, k * 512 + gg * 128:k * 512 + (gg + 1) * 128],
                    in_=tp[:],
                )
        dmas.append(dma)
        casts.append(cast)
        trans.append(tr)
    return dmas, casts, trans


def _proj_thunks(nc, pools, rhsT, wihT_get, bias_get, xgc, nw, pj_box, key):
    """Per-m thunk pairs: (3 matmuls into pj psum, DVE bias-add + fp8 cast).
    wihT_get/bias_get are callables resolved at emission time."""
    mm_thunks, cast_thunks = [], []
    for m in range(NM):
        def mm(m=m):
            wihT = wihT_get()
            pj = pools["pj"].tile([128, 512], F32, tag="pj")
            pj_box[(key, m)] = pj
            for k in range(EK):
                nc.tensor.matmul(
                    out=pj[:, :nw],
                    lhsT=wihT[:, k * G4 + m * 128:k * G4 + (m + 1) * 128],
                    rhs=rhsT[:, k * 512:k * 512 + nw],
                    start=(k == 0),
                    stop=(k == EK - 1),
                )

        def cast(m=m):
            pj = pj_box.pop((key, m))
            nc.vector.tensor_scalar(
                out=xgc[:, m, :nw], in0=pj[:, :nw],
                scalar1=bias_get()[:, m:m + 1], scalar2=None, op0=ALU.add,
            )
        mm_thunks.append(mm)
        cast_thunks.append(cast)
    return mm_thunks, cast_thunks


def _recurrence(nc, pools, whhT, xgcs, mask, c_init, blend_on, sel_tag, t_steps,
                ident8, filler, goff):
    """LSTM recurrence; returns running masked selection tile [128, 64].

    State layout: h^T/c^T as [128, 4*16]: partition r, col k*16+b holds
    state[k*128 + r, b].  Gate blocks (if / g / o) in double-buffered PSUM;
    per block one fp8 identity matmul injects the xg slice, then fp16
    W_hh matmuls accumulate on top.
    """
    sel_dt = F32 if blend_on == "c" else F16
    sel = pools["sel"].tile([128, NKH * BL], sel_dt, tag=sel_tag)
    nc.vector.memset(sel[:], 0.0)
    h16 = pools["h16"].tile([128, NKH * BL], F16, tag="h16")
    nc.vector.memset(h16[:], 0.0)
    if c_init is None:
        c = pools["cst"].tile([128, NKH * BL], F32, tag="cst")
        nc.vector.memset(c[:], 0.0)
    else:
        c = c_init

    blocks = ((0, 8, "ifps"), (8, 12, "gps"), (12, 16, "ops"))

    def alloc_ps():
        return {tag: pools[tag].tile([128, (m1 - m0) * BL], F32, tag=tag,
                                     name=tag)
                for m0, m1, tag in blocks}

    def prewrite(ps, t):
        """DVE-write the step-t xg slice into the gate-block PSUM tiles;
        the W_hh matmuls then accumulate on top (start=False)."""
        xgc = xgcs[t // SPC][:]
        off = (t % SPC) * BL
        for m0, m1, tag in blocks:
            nc.vector.tensor_copy(
                out=ps[tag][:].rearrange("p (m b) -> p m b", m=m1 - m0),
                in_=xgc[:, m0:m1, off:off + BL],
            )

    def pw(ps, tnext, m0, m1, tag):
        nc.vector.tensor_copy(
            out=ps[tag][:].rearrange("p (m b) -> p m b", m=m1 - m0),
            in_=xgcs[tnext // SPC][:][:, m0:m1, (tnext % SPC) * BL:
                                      (tnext % SPC) * BL + BL],
        )

    ps_cur = alloc_ps()
    prewrite(ps_cur, 0)
    for t in range(t_steps):
        ps = ps_cur
        last = t == t_steps - 1
        # next step's if-block xg prewrite first: with ifps bufs=3 its
        # buffer hazard resolved a full step ago, so it runs during the
        # matmul phase instead of inside the serial c-chain.
        if not last:
            ps_cur = alloc_ps()
            pw(ps_cur, t + 1, 0, 8, "ifps")
        for m0, m1, tag in blocks:
            p = ps[tag]
            for m in range(m0, m1):
                for k in range(NKH):
                    nc.tensor.matmul(
                        out=p[:, (m - m0) * BL:(m - m0 + 1) * BL],
                        lhsT=whhT[:, k * G4 + m * 128:k * G4 + (m + 1) * 128],
                        rhs=h16[:, k * BL:(k + 1) * BL],
                        start=False,
                        stop=(k == NKH - 1) and (m == m1 - 1),
                        skip_group_check=True,
                    )
        sig_if = pools["sig"].tile([128, 8 * BL], F32, tag="sig_if")
        nc.scalar.activation(out=sig_if[:], in_=ps["ifps"][:], func=AF.Sigmoid)
        tng = pools["t64"].tile([128, 64], F32, tag="tng")
        nc.scalar.activation(out=tng[:], in_=ps["gps"][:], func=AF.Tanh)
        tfc = pools["t64"].tile([128, 64], F32, tag="tfc")
        nc.vector.tensor_mul(out=tfc[:], in0=sig_if[:, 64:128], in1=c[:])
        tig = pools["t64"].tile([128, 64], F32, tag="tig")
        nc.vector.tensor_mul(out=tig[:], in0=sig_if[:, 0:64], in1=tng[:])
        c = pools["cst"].tile([128, NKH * BL], F32, tag="cst")
        nc.vector.tensor_add(out=c[:], in0=tfc[:], in1=tig[:])
        sgo = pools["t16"].tile([128, 64], F16, tag="sgo")
        nc.scalar.activation(out=sgo[:], in_=ps["ops"][:], func=AF.Sigmoid)
        tnc = pools["t16"].tile([128, 64], F16, tag="tnc")
        nc.scalar.activation(out=tnc[:], in_=c[:], func=AF.Tanh)
        h16 = pools["h16"].tile([128, NKH * BL], F16, tag="h16")
        nc.vector.tensor_mul(out=h16[:], in0=sgo[:], in1=tnc[:])

        # running masked blend (mask is exactly 0/1): sel = m ? src : sel
        src = c if blend_on == "c" else h16
        mslice = mask[:, t * BL:(t + 1) * BL]
        mbc = AP(mslice.tensor, mslice.offset, [mslice.ap[0], [0, NKH], [1, BL]])
        nc.vector.copy_predicated(
            out=sel[:].rearrange("p (j b) -> p j b", j=NKH),
            mask=mbc,
            data=src[:].rearrange("p (j b) -> p j b", j=NKH),
        )
        # g/o prewrites for t+1 at stream end: their ACT hazards (tng/sgo
        # of step t) completed mid-step, and the t+1 matmuls that read them
        # start 0.9us+ after h16, so these never stall anything.
        if not last:
            pw(ps_cur, t + 1, 8, 12, "gps")
            pw(ps_cur, t + 1, 12, 16, "ops")
        filler.drain(goff + t)
    return sel


def _head(nc, pools, sel_h, fcw_dram, fcb_dram, sim_dram, ident, out_dram):
    """logits[16,3] = [sel_h | sim | 1] @ [fc_W | fc_b]^T, then log_softmax."""
    fcw = pools["w32"].tile([C, H + 1], F32, tag="fcw")
    nc.sync.dma_start(out=fcw[:], in_=fcw_dram[:])
    fcwT = pools["persist"].tile([128, NKH * C], F16, tag="fcwT")
    for j in range(NKH):
        tp = pools["pj"].tile([128, C], F32, tag="pj")
        nc.tensor.transpose(
            out=tp[:], in_=fcw[:, j * 128:(j + 1) * 128], identity=ident[:C, :C]
        )
        nc.vector.tensor_copy(out=fcwT[:, j * C:(j + 1) * C], in_=tp[:])
    rhs45 = pools["persist"].tile([2, C], F32, tag="rhs45")
    nc.sync.dma_start(out=rhs45[0:1, :], in_=fcw_dram[:, H:H + 1].rearrange("a b -> b a"))
    nc.sync.dma_start(out=rhs45[1:2, :], in_=fcb_dram[None, :])
    lhsT45 = pools["persist"].tile([2, BL], F32, tag="lhsT45")
    nc.gpsimd.memset(lhsT45[:], 1.0)
    nc.sync.dma_start(out=lhsT45[0:1, :], in_=sim_dram.rearrange("a b -> b a"))

    lps = pools["pj"].tile([BL, C], F32, tag="pj")
    for j in range(NKH):
        nc.tensor.matmul(
            out=lps[:],
            lhsT=sel_h[:, j * BL:(j + 1) * BL],
            rhs=fcwT[:, j * C:(j + 1) * C],
            start=(j == 0),
            stop=False,
        )
    nc.tensor.matmul(out=lps[:], lhsT=lhsT45[:], rhs=rhs45[:], start=False, stop=True)

    mx = pools["head"].tile([BL, 1], F32, tag="mx")
    nc.vector.tensor_reduce(out=mx[:], in_=lps[:], axis=mybir.AxisListType.X, op=ALU.max)
    ls = pools["head"].tile([BL, C], F32, tag="ls")
    nc.vector.tensor_scalar(
        out=ls[:], in0=lps[:], scalar1=mx[:, 0:1], scalar2=None, op0=ALU.subtract
    )
    ex = pools["head"].tile([BL, C], F32, tag="ex")
    nc.scalar.activation(out=ex[:], in_=ls[:], func=AF.Exp)
    sm = pools["head"].tile([BL, 1], F32, tag="sm")
    nc.vector.tensor_reduce(out=sm[:], in_=ex[:], axis=mybir.AxisListType.X, op=ALU.add)
    lg = pools["head"].tile([BL, 1], F32, tag="lg")
    nc.scalar.activation(out=lg[:], in_=sm[:], func=AF.Ln)
    res = pools["head"].tile([BL, C], F32, tag="res")
    nc.vector.tensor_scalar(
        out=res[:], in0=ls[:], scalar1=lg[:, 0:1], scalar2=None, op0=ALU.subtract
    )
    nc.sync.dma_start(out=out_dram[:], in_=res[:])


def build(t_steps=T):
    nc = bacc.Bacc(
        "TRN2", target_bir_lowering=False, debug=False,
        enable_asserts=True, num_devices=NCORES,
    )
    prem = nc.declare_dram_parameter("premise", [BL, T], I32, isOutput=False)
    hyp = nc.declare_dram_parameter("hypothesis", [BL, T], I32, isOutput=False)
    sim = nc.declare_dram_parameter("similarity", [BL, 1], F32, isOutput=False)
    embw = nc.declare_dram_parameter("emb_weight", [V, E], F32, isOutput=False)
    wih_p = nc.declare_dram_parameter("Wih_p", [G4, E], F32, isOutput=False)
    whh_p = nc.declare_dram_parameter("Whh_p", [G4, H], F32, isOutput=False)
    bih_p = nc.declare_dram_parameter("bih_p", [G4], F32, isOutput=False)
    bhh_p = nc.declare_dram_parameter("bhh_p", [G4], F32, isOutput=False)
    wih_h = nc.declare_dram_parameter("Wih_h", [G4, E], F32, isOutput=False)
    whh_h = nc.declare_dram_parameter("Whh_h", [G4, H], F32, isOutput=False)
    bih_h = nc.declare_dram_parameter("bih_h", [G4], F32, isOutput=False)
    bhh_h = nc.declare_dram_parameter("bhh_h", [G4], F32, isOutput=False)
    fcw = nc.declare_dram_parameter("fc_W", [C, H + 1], F32, isOutput=False)
    fcb = nc.declare_dram_parameter("fc_b", [C], F32, isOutput=False)
    out = nc.declare_dram_parameter("out", [BL, C], F32, isOutput=True)

    with tile.TileContext(nc) as tc:
        from contextlib import ExitStack

        with ExitStack() as ctx:
            pools = {}

            def pool(name, bufs, space="SBUF"):
                pools[name] = ctx.enter_context(
                    tc.tile_pool(name=name, bufs=bufs, space=space)
                )

            pool("persist", 1)
            pool("w32", 4)
            pool("mscratch", 1)
            pool("mask", 1)
            pool("tok", 2)
            pool("emb32", 3)
            pool("emb16", 3)
            pool("rhsT", 3)
            pool("xgc", 1)
            pool("wihT", 2)
            pool("sel", 1)
            pool("h16", 2)
            pool("cst", 2)
            pool("sig", 2)
            pool("t64", 3)
            pool("t16", 2)
            pool("head", 1)
            pool("pj", 1, space="PSUM")
            pool("ifps", 3, space="PSUM")
            pool("gps", 2, space="PSUM")
            pool("ops", 2, space="PSUM")

            ident = pools["persist"].tile([128, 128], F32, tag="ident")
            make_identity(nc, ident[:])
            ident16 = pools["persist"].tile([128, 128], F16, tag="ident16")
            nc.vector.tensor_copy(out=ident16[:], in_=ident[:])
            ident8 = pools["persist"].tile([128, 128], F8, tag="ident8")
            nc.vector.tensor_copy(out=ident8[:], in_=ident[:])

            whhT_p = pools["persist"].tile([128, NKH * G4], F16, tag="whhT_p")
            whhT_h = pools["persist"].tile([128, NKH * G4], F16, tag="whhT_h")
            pj_box = {}

            # ---------- premise startup (serial): weights, chunk 0, mask ----
            tokt_p = pools["tok"].tile([128, NG], I32, tag="tok")
            _tok_dma(nc, tokt_p, prem)

            xgcs_p = [
                pools["xgc"].tile([128, NM, _chunk_ntok(ci)], F8,
                                  tag=f"xgc_p{ci}", name=f"xgc_p{ci}")
                for ci in range(NCH)
            ]
            xgcs_h = [
                pools["xgc"].tile([128, NM, _chunk_ntok(ci)], F8,
                                  tag=f"xgc_h{ci}", name=f"xgc_h{ci}")
                for ci in range(NCH)
            ]

            # chunk-0 gathers first so the indirect DMAs overlap weight loads
            rhsT0 = pools["rhsT"].tile([128, EK * 512], F16, tag="rhsT")
            dmas, casts, trans = _gather_chunk_thunks(
                nc, pools, tokt_p, embw, ident16, rhsT0, 0, 512)
            for fn in dmas:
                fn()
            wihT = pools["wihT"].tile([128, EK * G4], F16, tag="wihT")
            wload = _load_proj_weights(nc, pools, wih_p, wihT, ident)
            for fn in wload[:8]:
                fn()
            for fn in casts:
                fn()
            for fn in wload[8:]:
                fn()
            for fn in trans:
                fn()
            bias_p = _load_bias(nc, pools, bih_p[:], bhh_p[:], ident, "bias_p")
            mms, cst_t = _proj_thunks(
                nc, pools, rhsT0, lambda: wihT, lambda: bias_p,
                xgcs_p[0][:], 512, pj_box, "p0")
            wrec = _load_rec_weights(nc, pools, whh_p, whhT_p, ident)
            for i, (a, b) in enumerate(zip(mms, cst_t)):
                a()
                b()
                wrec[i]()
            mask_p = pools["mask"].tile([128, NTOK], U8, tag="mask_p")
            mthunks_p = _mask_chunk_thunks(nc, pools, prem, mask_p)
            mthunks_p[0]()

            # ---------- filler thunks (cost-paced, deadline-forced) ----
            filler = Filler()
            PACE = 0.68          # target filler us per step
            acc = [0.0]

            def pace(cost, minpos=0, dl=10**6):
                ms = max(int(acc[0] / PACE), minpos)
                acc[0] += cost
                return ms, dl

            def add_chunk(tokt, ci, nw, dl0, wihT_get, bias_get, xgc, key,
                          mask_thunk):
                rhsT_c = pools["rhsT"].tile([128, EK * 512], F16, tag="rhsT",
                                            name=f"rhsT_{key}")
                dmas, casts, trans = _gather_chunk_thunks(
                    nc, pools, tokt, embw, ident16, rhsT_c, ci, nw)
                ms0 = int(acc[0] / PACE)
                for fn in dmas:
                    ms, dl = pace(0.05, dl=dl0 - 14)
                    filler.add(ms, dl, 0.05, fn)
                for fn in casts:
                    ms, dl = pace(0.1, minpos=ms0 + 2, dl=dl0 - 12)
                    filler.add(ms, dl, 0.1, fn)
                for fn in trans:
                    ms, dl = pace(0.35, minpos=ms0 + 4, dl=dl0 - 10)
                    filler.add(ms, dl, 0.35, fn)
                mms, cst_t = _proj_thunks(
                    nc, pools, rhsT_c, wihT_get, bias_get, xgc, nw, pj_box, key)
                for i, (a, b) in enumerate(zip(mms, cst_t)):
                    ms, dl = pace(0.75, dl=dl0 - 6)
                    filler.add(ms, dl, 0.75, a)
                    filler.add(ms + 1, dl0 - 4, 0.3, b)
                    acc[0] += 0.3
                if mask_thunk is not None:
                    ms, dl = pace(0.3, dl=dl0 - 4)
                    filler.add(ms, dl, 0.3, mask_thunk)

            # premise chunks 1..6 (+ mask chunks), deadlines in premise steps
            for ci in range(1, NCH):
                add_chunk(tokt_p, ci, _chunk_ntok(ci), 32 * ci,
                          lambda: wihT, lambda: bias_p, xgcs_p[ci][:],
                          f"p{ci}", mthunks_p[ci])

            # hypothesis: weights, tokens, gather+projection, mask
            wihT2 = pools["wihT"].tile([128, EK * G4], F16, tag="wihT")
            for fn in _load_proj_weights(nc, pools, wih_h, wihT2, ident):
                ms, dl = pace(0.55, dl=186)
                filler.add(ms, dl, 0.55, fn)
            for fn in _load_rec_weights(nc, pools, whh_h, whhT_h, ident):
                ms, dl = pace(0.7, dl=196)
                filler.add(ms, dl, 0.7, fn)

            bias_h_box = []
            ms, dl = pace(0.2, dl=185)
            filler.add(ms, dl, 0.2, lambda: bias_h_box.append(
                _load_bias(nc, pools, bih_h[:], bhh_h[:], ident, "bias_h")))

            tokt_h = pools["tok"].tile([128, NG], I32, tag="tok")
            ms, dl = pace(0.05, dl=120)
            filler.add(ms, dl, 0.05, lambda: _tok_dma(nc, tokt_h, hyp))

            mask_h = pools["mask"].tile([128, NTOK], U8, tag="mask_h")
            mthunks_h = _mask_chunk_thunks(nc, pools, hyp, mask_h)
            ms, dl = pace(0.3, dl=195)
            filler.add(ms, dl, 0.3, mthunks_h[0])

            for ci in range(NCH):
                dl0 = 200 + 32 * ci if ci > 0 else 196
                add_chunk(tokt_h, ci, _chunk_ntok(ci), dl0,
                          lambda: wihT2, lambda: bias_h_box[0], xgcs_h[ci][:],
                          f"h{ci}", mthunks_h[ci] if ci > 0 else None)

            # ---------- premise recurrence (fillers drain into its gaps) ----
            sel_c = _recurrence(
                nc, pools, whhT_p, xgcs_p, mask_p, None, "c", "sel_c", t_steps,
                ident8, filler, 0,
            )

            # ---------- hypothesis recurrence ----
            sel_h = _recurrence(
                nc, pools, whhT_h, xgcs_h, mask_h, sel_c, "h", "sel_h", t_steps,
                ident8, filler, 200,
            )
            filler.drain_all()

            _head(nc, pools, sel_h, fcw, fcb, sim, ident, out)
    nc.compile()
    return nc


_NC_CACHE = {}


def _get_nc(t_steps=T):
    if t_steps not in _NC_CACHE:
        _NC_CACHE[t_steps] = build(t_steps)
    return _NC_CACHE[t_steps]


def kernel(**inputs):
    nc = _get_nc()
    prem = np.ascontiguousarray(np.asarray(inputs["premise"], dtype=np.int32))
    hyp = np.ascontiguousarray(np.asarray(inputs["hypothesis"], dtype=np.int32))
    sim = np.ascontiguousarray(np.asarray(inputs["similarity"], dtype=np.float32))
    shared = {
        name: np.ascontiguousarray(np.asarray(inputs[name], dtype=np.float32))
        for name in (
            "emb_weight", "Wih_p", "Whh_p", "bih_p", "bhh_p",
            "Wih_h", "Whh_h", "bih_h", "bhh_h", "fc_W", "fc_b",
        )
    }
    in_maps = []
    for i in range(NCORES):
        s = slice(i * BL, (i + 1) * BL)
        in_maps.append({"premise": prem[s], "hypothesis": hyp[s],
                        "similarity": sim[s], **shared})
    res = run_bass_kernel_spmd(nc, in_maps, list(range(NCORES)))
    return np.concatenate([res.results[i]["out"] for i in range(NCORES)], axis=0)


if __name__ == "__main__":
    rng = np.random.default_rng(0)
    ins = {
        "premise": rng.integers(0, V, (B, T)).astype(np.int32),
        "hypothesis": rng.integers(0, V, (B, T)).astype(np.int32),
        "similarity": rng.random((B, 1), dtype=np.float32),
        "emb_weight": rng.standard_normal((V, E), dtype=np.float32),
        "Wih_p": rng.standard_normal((G4, E), dtype=np.float32) * 0.04,
        "Whh_p": rng.standard_normal((G4, H), dtype=np.float32) * 0.04,
        "bih_p": rng.standard_normal(G4).astype(np.float32) * 0.04,
        "bhh_p": rng.standard_normal(G4).astype(np.float32) * 0.04,
        "Wih_h": rng.standard_normal((G4, E), dtype=np.float32) * 0.04,
        "Whh_h": rng.standard_normal((G4, H), dtype=np.float32) * 0.04,
        "bih_h": rng.standard_normal(G4).astype(np.float32) * 0.04,
        "bhh_h": rng.standard_normal(G4).astype(np.float32) * 0.04,
        "fc_W": rng.standard_normal((C, H + 1)).astype(np.float32) * 0.02,
        "fc_b": np.zeros(C, dtype=np.float32),
    }
    print(kernel(**ins).shape)
